# revision 6
# baseline (speedup 1.0000x reference)
"""Trainium2 Bass kernel for nn_MinimalAdderNN.

Computation (see reference): a 10-digit ripple-carry adder over base-10 digit
tensors a, b of shape [1048576, 10] (int32, digits 0..9), using two lookup
tables built deterministically by setup_inputs(). For those structured tables
the output rows are exact one-hots:
    out[n, 1+p, k] = (k == (a[n,p] + b[n,p] + carry_in) % 10)
    out[n, 0,   k] = (k == final_carry)

v2 pipeline (per core, MSD-first layout so no reversed broadcasts):
    host packs p = a | b<<4 (uint8, halves the input HBM reads)
    DVE:    hi8 = p >> 4
    Pool:   s = hi8 * -15 + p      (= a + b, f32), pad slot = 0
    DVE:    t = scan(10 is_le state; add s) over reversed views
            -> t[w] = s[w] + carry_in, pad resets the chain per element
    DVE:    c10 = (t is_ge 10) * -10 ; d[1+w] = t[w] + c10[w] ; d[0] = lead
    one-hot: DVE is_equal for slots 0..V1-1; Pool diff + ScalarE Square/Relu
            for slots V1..10, all written straight to the output layout.

Sharding: pure data-parallel over batch across 8 NeuronCores (131072 rows
per core); the tables are consumed host-side only (validated against the
expected structured tables).
"""
import sys

sys.path.insert(0, "/opt/trn_rl_repo")

import numpy as np

import concourse.bacc as bacc
import concourse.mybir as mybir
import concourse.tile as tile
from concourse.bass_utils import run_bass_kernel_spmd

BATCH = 1048576
D = 10
NCORES = 8
NPC = BATCH // NCORES  # 131072 rows per core
P = 128
# Variable tile sizes (batch elems per partition per tile): small head/tail
# tiles shorten pipeline fill/drain; must sum to NPC // P = 1024.
QS = [32, 48] + [72] * 12 + [48, 32]
PW = D + 1             # padded slots per batch elem
OW = (D + 1) * D       # 110 output floats per batch elem
V1 = 4                 # one-hot slots 0..V1-1 on DVE; rest via Pool+ScalarE

f32 = mybir.dt.float32
bf16 = mybir.dt.bfloat16
u8 = mybir.dt.uint8
A = mybir.AluOpType

_CACHE = {}


def _expected_tables():
    next_carry = np.zeros((200, 2), dtype=np.float32)
    digit = np.zeros((200, 10), dtype=np.float32)
    for carry in (0, 1):
        for a_ in range(10):
            for b_ in range(10):
                idx = carry * 100 + a_ * 10 + b_
                total = a_ + b_ + carry
                next_carry[idx, total // 10] = 1.0
                digit[idx, total % 10] = 1.0
    return digit, next_carry


def _tables_are_structured(digit_table, carry_table):
    digit_exp, carry_exp = _expected_tables()
    if digit_table.shape != (200, 10) or carry_table.shape != (200, 2):
        return False
    if not np.array_equal(digit_table, digit_exp):
        return False
    # The reference only consumes argmax(carry_table[idx]); the fast path is
    # valid iff that argmax equals the arithmetic carry bit for every index.
    bits = np.argmax(carry_table, axis=1)
    return np.array_equal(bits, np.argmax(carry_exp, axis=1))


def _build_fast_nc():
    assert sum(QS) * P == NPC
    qmax = max(QS)
    SS = PW - V1  # scalar-engine one-hot slots
    nc = bacc.Bacc()
    pk_d = nc.dram_tensor("pk", [NPC, D], u8, kind="ExternalInput").ap()
    o_d = nc.dram_tensor("out", [NPC, OW], f32, kind="ExternalOutput").ap()

    with tile.TileContext(nc) as tc:
        with tc.tile_pool(name="const", bufs=1) as cp, \
             tc.tile_pool(name="io", bufs=8) as iop, \
             tc.tile_pool(name="wk", bufs=2) as wp, \
             tc.tile_pool(name="ot", bufs=4) as op_:
            iota = cp.tile([P, OW], f32, tag="iota")
            nc.gpsimd.iota(iota[:], pattern=[[0, PW], [1, D]], base=0,
                           channel_multiplier=0,
                           allow_small_or_imprecise_dtypes=True)
            i3 = iota[:].rearrange("p (w k) -> p w k", k=D)
            tens = cp.tile([P, qmax * PW], f32, tag="tens")
            nc.vector.memset(tens[:], 10.0)

            r0 = 0
            for t_i, Q in enumerate(QS):
                pk_src = pk_d[r0:r0 + P * Q, :] \
                    .rearrange("(p q) d -> p (q d)", q=Q)
                o_dst = o_d[r0:r0 + P * Q, :] \
                    .rearrange("(p q) d -> p (q d)", q=Q)
                r0 += P * Q

                pt = iop.tile([P, qmax * D], u8, tag="pk")
                nc.scalar.dma_start(pt[:, :Q * D], pk_src)
                p3 = pt[:, :Q * D].rearrange("p (q d) -> p q d", d=D)

                hi = wp.tile([P, qmax * D], u8, tag="hi")
                nc.vector.tensor_scalar(hi[:, :Q * D], pt[:, :Q * D], 4, None,
                                        op0=A.logical_shift_right)
                h3 = hi[:, :Q * D].rearrange("p (q d) -> p q d", d=D)

                # s = a + b = hi*-15 + p  (p = a + 16 b, hi = b)
                s = wp.tile([P, qmax * PW], f32, tag="s")
                s3 = s[:, :Q * PW].rearrange("p (q w) -> p q w", w=PW)
                nc.gpsimd.memset(s3[:, :, D:PW], 0.0)
                nc.vector.scalar_tensor_tensor(s3[:, :, 0:D], h3, -15.0, p3,
                                               op0=A.mult, op1=A.add)

                # t[w] = s[w] + carry_in, LSD-first chain via reversed views
                tt = wp.tile([P, qmax * PW], f32, tag="t")
                nc.vector.tensor_tensor_scan(
                    tt[:, :Q * PW][:, ::-1], tens[:, :Q * PW][:, ::-1],
                    s[:, :Q * PW][:, ::-1], 0.0,
                    op0=A.is_le, op1=A.add)
                t3 = tt[:, :Q * PW].rearrange("p (q w) -> p q w", w=PW)

                c10 = wp.tile([P, qmax * D], f32, tag="c10")
                c3 = c10[:, :Q * D].rearrange("p (q d) -> p q d", d=D)
                nc.vector.tensor_scalar(c3, t3[:, :, 0:D], 10.0, -10.0,
                                        op0=A.is_ge, op1=A.mult)

                d = wp.tile([P, qmax * PW], f32, tag="d")
                d3 = d[:, :Q * PW].rearrange("p (q w) -> p q w", w=PW)
                nc.vector.tensor_tensor(d3[:, :, 1:PW], t3[:, :, 0:D], c3,
                                        op=A.add)
                nc.vector.tensor_scalar(d3[:, :, 0:1], t3[:, :, 0:1], 10.0,
                                        None, op0=A.is_ge)

                ot = op_.tile([P, qmax * OW], f32, tag="o")
                o4 = ot[:, :Q * OW].rearrange("p (q w k) -> p q w k",
                                              w=PW, k=D)
                d_bc = d3.unsqueeze(3).broadcast_to([P, Q, PW, D])
                i4 = i3.unsqueeze(1).broadcast_to([P, Q, PW, D])
                nc.vector.tensor_tensor(o4[:, :, 0:V1, :], d_bc[:, :, 0:V1],
                                        i4[:, :, 0:V1], op=A.is_equal)

                df = wp.tile([P, qmax * SS * D], bf16, tag="df")
                df4 = df[:, :Q * SS * D].rearrange("p (q w k) -> p q w k",
                                                   w=SS, k=D)
                nc.gpsimd.tensor_tensor(df4, d_bc[:, :, V1:PW],
                                        i4[:, :, V1:PW], op=A.subtract)
                df2 = df[:, :Q * SS * D]
                nc.scalar.activation(df2, df2,
                                     mybir.ActivationFunctionType.Square,
                                     bias=0.0, scale=1.0)
                nc.scalar.activation(o4[:, :, V1:PW, :], df4,
                                     mybir.ActivationFunctionType.Relu,
                                     bias=1.0, scale=-1.0)

                nc.sync.dma_start(o_dst, ot[:, :Q * OW])
    nc.compile()
    return nc


def _run_fast(a, b, trace=False, trace_kwargs=None):
    if "fast_nc" not in _CACHE:
        _CACHE["fast_nc"] = _build_fast_nc()
    nc = _CACHE["fast_nc"]
    packed = (a.astype(np.uint8) | (b.astype(np.uint8) << 4))
    in_maps = []
    for cid in range(NCORES):
        sl = slice(cid * NPC, (cid + 1) * NPC)
        in_maps.append({"pk": np.ascontiguousarray(packed[sl])})
    res = run_bass_kernel_spmd(nc, in_maps, core_ids=list(range(NCORES)),
                               trace=trace, **(trace_kwargs or {}))
    out = np.concatenate([r["out"] for r in res.results], axis=0)
    return out.reshape(BATCH, D + 1, D), res


def _run_general_host(a, b, digit_table, carry_table):
    # Correctness fallback for non-structured tables (not expected from the
    # reference's setup_inputs); computed host-side.
    n = a.shape[0]
    carry = np.zeros(n, dtype=np.int64)
    out = np.empty((n, D + 1, D), dtype=digit_table.dtype)
    for p in range(D - 1, -1, -1):
        idx = carry * 100 + a[:, p].astype(np.int64) * 10 + b[:, p].astype(np.int64)
        out[:, 1 + p, :] = digit_table[idx]
        carry = np.argmax(carry_table[idx], axis=1)
    lead = np.zeros((n, D), dtype=digit_table.dtype)
    lead[np.arange(n), carry] = 1.0
    out[:, 0, :] = lead
    return out


def kernel(a, b, digit_table, carry_table):
    a = np.asarray(a, dtype=np.int32)
    b = np.asarray(b, dtype=np.int32)
    digit_table = np.asarray(digit_table, dtype=np.float32)
    carry_table = np.asarray(carry_table, dtype=np.float32)
    assert a.shape == (BATCH, D) and b.shape == (BATCH, D), (a.shape, b.shape)
    if _tables_are_structured(digit_table, carry_table):
        out, _ = _run_fast(a, b)
        return out
    return _run_general_host(a, b, digit_table, carry_table)


# revision 9
# speedup vs baseline: 1.2359x; 1.2359x over previous
"""Trainium2 Bass kernel for nn_MinimalAdderNN.

Computation (see reference): a 10-digit ripple-carry adder over base-10 digit
tensors a, b of shape [1048576, 10] (int32, digits 0..9), using two lookup
tables built deterministically by setup_inputs(). For those structured tables
the output rows are exact one-hots:
    out[n, 1+p, k] = (k == (a[n,p] + b[n,p] + carry_in) % 10)
    out[n, 0,   k] = (k == final_carry)

v3 pipeline (per core, MSD-first layout so no reversed broadcasts; the
whole chain runs in bf16 — every value is a small integer, exact in bf16 —
and the output store is a SWDGE DMA that casts bf16 -> f32 on the fly, so
HBM write bytes are unchanged but engine and SBUF cost halve):
    host packs p = a | b<<4 (uint8, halves the input HBM reads)
    DVE:    hi8 = p >> 4 ; s = hi8 * -15 + p   (= a + b), pad slot = 0
    DVE:    t = scan(10 is_le state; add s) over reversed views
            -> t[w] = s[w] + carry_in, pad resets the chain per element
    DVE:    c10 = (t is_ge 10) * -10 ; d[1+w] = t[w] + c10[w] ; d[0] = lead
    one-hot: DVE is_equal for slots 0..V1-1; ScalarE per-class Square +
            one Relu for slots V1..10, all written to the output layout.

Sharding: pure data-parallel over batch across 8 NeuronCores (131072 rows
per core); the tables are consumed host-side only (validated against the
expected structured tables).
"""
import sys

sys.path.insert(0, "/opt/trn_rl_repo")

import numpy as np

import concourse.bacc as bacc
import concourse.mybir as mybir
import concourse.tile as tile
from concourse.bass_utils import run_bass_kernel_spmd

BATCH = 1048576
D = 10
NCORES = 8
NPC = BATCH // NCORES  # 131072 rows per core
P = 128
# Variable tile sizes (batch elems per partition per tile): small head/tail
# tiles shorten pipeline fill/drain; must sum to NPC // P = 1024.
QS = [16, 48, 96] + [128] * 6 + [64, 32]
PW = D + 1             # padded slots per batch elem
OW = (D + 1) * D       # 110 output floats per batch elem
V1 = 8                 # one-hot slots 0..V1-1 on DVE; rest on ScalarE

f32 = mybir.dt.float32
bf16 = mybir.dt.bfloat16
u8 = mybir.dt.uint8
A = mybir.AluOpType

_CACHE = {}


def _expected_tables():
    next_carry = np.zeros((200, 2), dtype=np.float32)
    digit = np.zeros((200, 10), dtype=np.float32)
    for carry in (0, 1):
        for a_ in range(10):
            for b_ in range(10):
                idx = carry * 100 + a_ * 10 + b_
                total = a_ + b_ + carry
                next_carry[idx, total // 10] = 1.0
                digit[idx, total % 10] = 1.0
    return digit, next_carry


def _tables_are_structured(digit_table, carry_table):
    digit_exp, carry_exp = _expected_tables()
    if digit_table.shape != (200, 10) or carry_table.shape != (200, 2):
        return False
    if not np.array_equal(digit_table, digit_exp):
        return False
    # The reference only consumes argmax(carry_table[idx]); the fast path is
    # valid iff that argmax equals the arithmetic carry bit for every index.
    bits = np.argmax(carry_table, axis=1)
    return np.array_equal(bits, np.argmax(carry_exp, axis=1))


def _build_fast_nc():
    assert sum(QS) * P == NPC
    qmax = max(QS)
    SS = PW - V1  # scalar-engine one-hot slots
    nc = bacc.Bacc()
    pk_d = nc.dram_tensor("pk", [NPC, D], u8, kind="ExternalInput").ap()
    o_d = nc.dram_tensor("out", [NPC, OW], f32, kind="ExternalOutput").ap()

    with tile.TileContext(nc) as tc:
        with tc.tile_pool(name="const", bufs=1) as cp, \
             tc.tile_pool(name="io", bufs=8) as iop, \
             tc.tile_pool(name="wk", bufs=2) as wp, \
             tc.tile_pool(name="ot", bufs=4) as op_:
            iota = cp.tile([P, OW], bf16, tag="iota")
            nc.gpsimd.iota(iota[:], pattern=[[0, PW], [1, D]], base=0,
                           channel_multiplier=0,
                           allow_small_or_imprecise_dtypes=True)
            i3 = iota[:].rearrange("p (w k) -> p w k", k=D)
            tens = cp.tile([P, qmax * PW], bf16, tag="tens")
            nc.vector.memset(tens[:], 10.0)
            bias_t = cp.tile([P, D], bf16, tag="bias")
            for k in range(D):
                nc.vector.memset(bias_t[:, k:k + 1], -float(k))

            r0 = 0
            for t_i, Q in enumerate(QS):
                pk_src = pk_d[r0:r0 + P * Q, :] \
                    .rearrange("(p q) d -> p (q d)", q=Q)
                o_dst = o_d[r0:r0 + P * Q, :] \
                    .rearrange("(p q) d -> p (q d)", q=Q)
                r0 += P * Q

                pt = iop.tile([P, qmax * D], u8, tag="pk")
                nc.scalar.dma_start(pt[:, :Q * D], pk_src)
                p3 = pt[:, :Q * D].rearrange("p (q d) -> p q d", d=D)

                hi = wp.tile([P, qmax * D], u8, tag="hi")
                nc.vector.tensor_scalar(hi[:, :Q * D], pt[:, :Q * D], 4, None,
                                        op0=A.logical_shift_right)
                h3 = hi[:, :Q * D].rearrange("p (q d) -> p q d", d=D)

                # s = a + b = hi*-15 + p  (p = a + 16 b, hi = b)
                s = wp.tile([P, qmax * PW], bf16, tag="s")
                s3 = s[:, :Q * PW].rearrange("p (q w) -> p q w", w=PW)
                nc.gpsimd.memset(s3[:, :, D:PW], 0.0)
                nc.vector.scalar_tensor_tensor(s3[:, :, 0:D], h3, -15.0, p3,
                                               op0=A.mult, op1=A.add)

                # t[w] = s[w] + carry_in, LSD-first chain via reversed views
                tt = wp.tile([P, qmax * PW], bf16, tag="t")
                nc.vector.tensor_tensor_scan(
                    tt[:, :Q * PW][:, ::-1], tens[:, :Q * PW][:, ::-1],
                    s[:, :Q * PW][:, ::-1], 0.0,
                    op0=A.is_le, op1=A.add)
                t3 = tt[:, :Q * PW].rearrange("p (q w) -> p q w", w=PW)

                c10 = wp.tile([P, qmax * D], bf16, tag="c10")
                c3 = c10[:, :Q * D].rearrange("p (q d) -> p q d", d=D)
                nc.vector.tensor_scalar(c3, t3[:, :, 0:D], 10.0, -10.0,
                                        op0=A.is_ge, op1=A.mult)

                d = wp.tile([P, qmax * PW], bf16, tag="d")
                d3 = d[:, :Q * PW].rearrange("p (q w) -> p q w", w=PW)
                nc.vector.tensor_tensor(d3[:, :, 1:PW], t3[:, :, 0:D], c3,
                                        op=A.add)
                nc.vector.tensor_scalar(d3[:, :, 0:1], t3[:, :, 0:1], 10.0,
                                        None, op0=A.is_ge)

                ot = op_.tile([P, qmax * OW], bf16, tag="o")
                o4 = ot[:, :Q * OW].rearrange("p (q w k) -> p q w k",
                                              w=PW, k=D)
                d_bc = d3.unsqueeze(3).broadcast_to([P, Q, PW, D])
                i4 = i3.unsqueeze(1).broadcast_to([P, Q, PW, D])
                nc.vector.tensor_tensor(o4[:, :, 0:V1, :], d_bc[:, :, 0:V1],
                                        i4[:, :, 0:V1], op=A.is_equal)

                # slots V1..10 on ScalarE: per-class (d-k)^2 then relu(1-sq)
                sq = wp.tile([P, qmax * SS * D], bf16, tag="sq")
                sq4 = sq[:, :Q * SS * D].rearrange("p (q w k) -> p q w k",
                                                   w=SS, k=D)
                for k in range(D):
                    nc.scalar.activation(
                        sq4[:, :, :, k:k + 1].squeeze(3),
                        d3[:, :, V1:PW],
                        mybir.ActivationFunctionType.Square,
                        bias=bias_t[:, k:k + 1], scale=1.0)
                nc.scalar.activation(o4[:, :, V1:PW, :], sq4,
                                     mybir.ActivationFunctionType.Relu,
                                     bias=1.0, scale=-1.0)

                nc.gpsimd.dma_start(o_dst, ot[:, :Q * OW])
    nc.compile()
    return nc


def _run_fast(a, b, trace=False, trace_kwargs=None):
    if "fast_nc" not in _CACHE:
        _CACHE["fast_nc"] = _build_fast_nc()
    nc = _CACHE["fast_nc"]
    packed = (a.astype(np.uint8) | (b.astype(np.uint8) << 4))
    in_maps = []
    for cid in range(NCORES):
        sl = slice(cid * NPC, (cid + 1) * NPC)
        in_maps.append({"pk": np.ascontiguousarray(packed[sl])})
    res = run_bass_kernel_spmd(nc, in_maps, core_ids=list(range(NCORES)),
                               trace=trace, **(trace_kwargs or {}))
    out = np.concatenate([r["out"] for r in res.results], axis=0)
    return out.reshape(BATCH, D + 1, D), res


def _run_general_host(a, b, digit_table, carry_table):
    # Correctness fallback for non-structured tables (not expected from the
    # reference's setup_inputs); computed host-side.
    n = a.shape[0]
    carry = np.zeros(n, dtype=np.int64)
    out = np.empty((n, D + 1, D), dtype=digit_table.dtype)
    for p in range(D - 1, -1, -1):
        idx = carry * 100 + a[:, p].astype(np.int64) * 10 + b[:, p].astype(np.int64)
        out[:, 1 + p, :] = digit_table[idx]
        carry = np.argmax(carry_table[idx], axis=1)
    lead = np.zeros((n, D), dtype=digit_table.dtype)
    lead[np.arange(n), carry] = 1.0
    out[:, 0, :] = lead
    return out


def kernel(a, b, digit_table, carry_table):
    a = np.asarray(a, dtype=np.int32)
    b = np.asarray(b, dtype=np.int32)
    digit_table = np.asarray(digit_table, dtype=np.float32)
    carry_table = np.asarray(carry_table, dtype=np.float32)
    assert a.shape == (BATCH, D) and b.shape == (BATCH, D), (a.shape, b.shape)
    if _tables_are_structured(digit_table, carry_table):
        out, _ = _run_fast(a, b)
        return out
    return _run_general_host(a, b, digit_table, carry_table)


# revision 10
# speedup vs baseline: 1.3533x; 1.0950x over previous
"""Trainium2 Bass kernel for nn_MinimalAdderNN.

Computation (see reference): a 10-digit ripple-carry adder over base-10 digit
tensors a, b of shape [1048576, 10] (int32, digits 0..9), using two lookup
tables built deterministically by setup_inputs(). For those structured tables
the output rows are exact one-hots:
    out[n, 1+p, k] = (k == (a[n,p] + b[n,p] + carry_in) % 10)
    out[n, 0,   k] = (k == final_carry)

v4 pipeline (per core, MSD-first layout; the whole chain runs in bf16 —
every value is a small integer, exact in bf16 — and the output store is a
SWDGE DMA that casts bf16 -> f32 on the fly, so HBM write bytes are
unchanged but engine time and SBUF traffic halve):
    host packs p = a | b<<4 (uint8, halves the input HBM reads)
    DVE:    hi8 = p >> 4 ; s[1+j] = hi8 * -15 + p  (= a + b); s[0] = pad 0
    DVE:    t = scan(10 is_le state; add s) over reversed views
            -> t[1+j] = s digit + carry_in; the pad slot lands the FINAL
            carry at w=0 (the lead slot), and resets the chain per element
    DVE:    c10 = (t is_ge 10) * -10 ; d = t + c10   (full-width, 2D)
            slot 0: carry < 10 so c10 = 0 and d[0] = final carry, for free
    one-hot: single DVE is_equal over all 11 slots vs the iota table.
All compute is on the VectorEngine (~17 us/tile vs ~20 us/tile of DMA);
ScalarE only triggers input DMAs, GpSimd only memsets + output DMAs.

Sharding: pure data-parallel over batch across 8 NeuronCores (131072 rows
per core); the tables are consumed host-side only (validated against the
expected structured tables).
"""
import sys

sys.path.insert(0, "/opt/trn_rl_repo")

import numpy as np

import concourse.bacc as bacc
import concourse.mybir as mybir
import concourse.tile as tile
from concourse.bass_utils import run_bass_kernel_spmd

BATCH = 1048576
D = 10
NCORES = 8
NPC = BATCH // NCORES  # 131072 rows per core
P = 128
# Variable tile sizes (batch elems per partition per tile): small head/tail
# tiles shorten pipeline fill/drain; must sum to NPC // P = 1024.
QS = [16, 48, 96] + [128] * 6 + [64, 32]
PW = D + 1             # padded slots per batch elem
OW = (D + 1) * D       # 110 output floats per batch elem


f32 = mybir.dt.float32
bf16 = mybir.dt.bfloat16
u8 = mybir.dt.uint8
A = mybir.AluOpType

_CACHE = {}


def _expected_tables():
    next_carry = np.zeros((200, 2), dtype=np.float32)
    digit = np.zeros((200, 10), dtype=np.float32)
    for carry in (0, 1):
        for a_ in range(10):
            for b_ in range(10):
                idx = carry * 100 + a_ * 10 + b_
                total = a_ + b_ + carry
                next_carry[idx, total // 10] = 1.0
                digit[idx, total % 10] = 1.0
    return digit, next_carry


def _tables_are_structured(digit_table, carry_table):
    digit_exp, carry_exp = _expected_tables()
    if digit_table.shape != (200, 10) or carry_table.shape != (200, 2):
        return False
    if not np.array_equal(digit_table, digit_exp):
        return False
    # The reference only consumes argmax(carry_table[idx]); the fast path is
    # valid iff that argmax equals the arithmetic carry bit for every index.
    bits = np.argmax(carry_table, axis=1)
    return np.array_equal(bits, np.argmax(carry_exp, axis=1))


def _build_fast_nc():
    assert sum(QS) * P == NPC
    qmax = max(QS)
    nc = bacc.Bacc()
    pk_d = nc.dram_tensor("pk", [NPC, D], u8, kind="ExternalInput").ap()
    o_d = nc.dram_tensor("out", [NPC, OW], f32, kind="ExternalOutput").ap()

    with tile.TileContext(nc) as tc:
        with tc.tile_pool(name="const", bufs=1) as cp, \
             tc.tile_pool(name="io", bufs=8) as iop, \
             tc.tile_pool(name="wk", bufs=2) as wp, \
             tc.tile_pool(name="ot", bufs=4) as op_:
            iota = cp.tile([P, OW], bf16, tag="iota")
            nc.gpsimd.iota(iota[:], pattern=[[0, PW], [1, D]], base=0,
                           channel_multiplier=0,
                           allow_small_or_imprecise_dtypes=True)
            i3 = iota[:].rearrange("p (w k) -> p w k", k=D)
            tens = cp.tile([P, qmax * PW], bf16, tag="tens")
            nc.vector.memset(tens[:], 10.0)

            r0 = 0
            for t_i, Q in enumerate(QS):
                pk_src = pk_d[r0:r0 + P * Q, :] \
                    .rearrange("(p q) d -> p (q d)", q=Q)
                o_dst = o_d[r0:r0 + P * Q, :] \
                    .rearrange("(p q) d -> p (q d)", q=Q)
                r0 += P * Q

                pt = iop.tile([P, qmax * D], u8, tag="pk")
                nc.scalar.dma_start(pt[:, :Q * D], pk_src)
                p3 = pt[:, :Q * D].rearrange("p (q d) -> p q d", d=D)

                hi = wp.tile([P, qmax * D], u8, tag="hi")
                nc.vector.tensor_scalar(hi[:, :Q * D], pt[:, :Q * D], 4, None,
                                        op0=A.logical_shift_right)
                h3 = hi[:, :Q * D].rearrange("p (q d) -> p q d", d=D)

                # s = a + b = hi*-15 + p  (p = a + 16 b, hi = b); pad at w=0
                s = wp.tile([P, qmax * PW], bf16, tag="s")
                s3 = s[:, :Q * PW].rearrange("p (q w) -> p q w", w=PW)
                nc.gpsimd.memset(s3[:, :, 0:1], 0.0)
                nc.vector.scalar_tensor_tensor(s3[:, :, 1:PW], h3, -15.0, p3,
                                               op0=A.mult, op1=A.add)

                # t[1+j] = s[1+j] + carry_in (LSD-first chain via reversed
                # views); t[0] = carry out of the MSD = the lead digit.
                tt = wp.tile([P, qmax * PW], bf16, tag="t")
                nc.vector.tensor_tensor_scan(
                    tt[:, :Q * PW][:, ::-1], tens[:, :Q * PW][:, ::-1],
                    s[:, :Q * PW][:, ::-1], 0.0,
                    op0=A.is_le, op1=A.add)
                t2 = tt[:, :Q * PW]

                c10 = wp.tile([P, qmax * PW], bf16, tag="c10")
                c2 = c10[:, :Q * PW]
                nc.vector.tensor_scalar(c2, t2, 10.0, -10.0,
                                        op0=A.is_ge, op1=A.mult)

                d = wp.tile([P, qmax * PW], bf16, tag="d")
                nc.vector.tensor_tensor(d[:, :Q * PW], t2, c2, op=A.add)
                d3 = d[:, :Q * PW].rearrange("p (q w) -> p q w", w=PW)

                ot = op_.tile([P, qmax * OW], bf16, tag="o")
                o4 = ot[:, :Q * OW].rearrange("p (q w k) -> p q w k",
                                              w=PW, k=D)
                d_bc = d3.unsqueeze(3).broadcast_to([P, Q, PW, D])
                i4 = i3.unsqueeze(1).broadcast_to([P, Q, PW, D])
                nc.vector.tensor_tensor(o4, d_bc, i4, op=A.is_equal)

                nc.gpsimd.dma_start(o_dst, ot[:, :Q * OW])
    nc.compile()
    return nc


def _run_fast(a, b, trace=False, trace_kwargs=None):
    if "fast_nc" not in _CACHE:
        _CACHE["fast_nc"] = _build_fast_nc()
    nc = _CACHE["fast_nc"]
    packed = (a.astype(np.uint8) | (b.astype(np.uint8) << 4))
    in_maps = []
    for cid in range(NCORES):
        sl = slice(cid * NPC, (cid + 1) * NPC)
        in_maps.append({"pk": np.ascontiguousarray(packed[sl])})
    res = run_bass_kernel_spmd(nc, in_maps, core_ids=list(range(NCORES)),
                               trace=trace, **(trace_kwargs or {}))
    out = np.concatenate([r["out"] for r in res.results], axis=0)
    return out.reshape(BATCH, D + 1, D), res


def _run_general_host(a, b, digit_table, carry_table):
    # Correctness fallback for non-structured tables (not expected from the
    # reference's setup_inputs); computed host-side.
    n = a.shape[0]
    carry = np.zeros(n, dtype=np.int64)
    out = np.empty((n, D + 1, D), dtype=digit_table.dtype)
    for p in range(D - 1, -1, -1):
        idx = carry * 100 + a[:, p].astype(np.int64) * 10 + b[:, p].astype(np.int64)
        out[:, 1 + p, :] = digit_table[idx]
        carry = np.argmax(carry_table[idx], axis=1)
    lead = np.zeros((n, D), dtype=digit_table.dtype)
    lead[np.arange(n), carry] = 1.0
    out[:, 0, :] = lead
    return out


def kernel(a, b, digit_table, carry_table):
    a = np.asarray(a, dtype=np.int32)
    b = np.asarray(b, dtype=np.int32)
    digit_table = np.asarray(digit_table, dtype=np.float32)
    carry_table = np.asarray(carry_table, dtype=np.float32)
    assert a.shape == (BATCH, D) and b.shape == (BATCH, D), (a.shape, b.shape)
    if _tables_are_structured(digit_table, carry_table):
        out, _ = _run_fast(a, b)
        return out
    return _run_general_host(a, b, digit_table, carry_table)
